# revision 8
# baseline (speedup 1.0000x reference)
"""Multi-Head Latent Attention Trainium2 kernel (8-core data parallel).

Sharding: pure data parallel over (batch=4) x (sequence halves=2) = 8 cores.
Each core computes the full attention output for its 1024 query tokens of one
batch, using all 2048 keys/values of that batch (KV computed redundantly per
batch pair — negligible cost). No collectives.

Per-core kernel (matmul cost = moving rows; f32r 1 cyc/row at free>=256):
  0. inputs land via few BIG DMAs (host pre-lays x/weights in partition-major
     order) — DMA issue on the sync queue costs ~2us each, so count matters.
  1. latent.T [128, 2048] = [w_kv_a; w_q_a] @ x.T; rmsnorm via ones-matmul
     sum-of-squares, inv_rms = exp(-0.5*ln(ms+eps)) on ACT (ops batched
     per-function so each activation table loads once); partition broadcast
     of inv_rms via a sel2 matmul into psum (no DMA round trip).
  2. V per GROUP of two head-pairs: [128tok, 256] f32r matmuls (1 cyc/row).
     Per head pair (A=2hp rows 0-63, B=2hp+1 rows 64-127):
     S^T = K.T_tile.T @ Q.T chunk (A/B row groups, shared 2-bank psum),
     P = exp(S*scale) one ACT op per [128,1024] tile, y_aug.T accumulated
     over k-tiles (M=65 incl. denominator row), y matmuls delayed two
     k-tiles so the PE never blocks on ACT. Softmax normalization:
     reciprocal in place on the denominator row (partition 64), broadcast
     down 64 partitions with a ones-row matmul (bases aligned), DVE multiply.
  3. proj: spread through the attention phase as filler work: per
     (head-pair, qt, cc) two psum-accumulated matmuls + one DVE accumulate
     into osum; per-(qt,cc) output DMA after the pair=3 partial. For hp=7
     the qc order is swapped so only one proj batch remains after the last
     chunk.
  ACT runs ONLY the exps (the ~290us/core softmax exp is the hard floor:
  no other engine has exp); all PSUM evacuations (z/V/ysb/ostrip) run on
  DVE, which also does reciprocals, normalize multiplies and osum
  accumulation; Pool issues weight/output DMAs.

Token order per core: [own 1024 queries, other half] so the SPMD NEFF always
reads queries at offset 0 (K/V order irrelevant to softmax).
"""
from collections import deque

import numpy as np
import ml_dtypes

import concourse.bacc as bacc
import concourse.bass as bass
import concourse.mybir as mybir
import concourse.tile as tile
from concourse.bass_utils import run_bass_kernel_spmd

F32 = mybir.dt.float32
F32R = mybir.dt.float32r
BF16 = mybir.dt.bfloat16
AF = mybir.ActivationFunctionType

B, N, C = 4, 2048, 1024
H, D, R = 16, 64, 64
NT = 2048          # kv tokens per core (full batch sequence)
NQ = 1024          # query tokens per core
EPS = 1e-6
SCALE = D ** -0.5
N_CORES = 8


def build_nc(reps: int = 1, ablate=()):
    ablate = set(ablate)
    nc = bacc.Bacc("TRN2", target_bir_lowering=False)
    # partition-major host layouts: one DMA each
    x_t = nc.dram_tensor("x_t", [128, 8, NT], BF16, kind="ExternalInput")
    wa_t = nc.dram_tensor("wa_t", [128, 8, 2 * R], BF16, kind="ExternalInput")
    m_t = nc.dram_tensor("m_t", [128, 8, 128], F32R, kind="ExternalInput")
    wv_t = nc.dram_tensor("wv_t", [128, H * D], F32R, kind="ExternalInput")
    wp_t = nc.dram_tensor("wp_t", [128, 8, C], BF16, kind="ExternalInput")
    ones2_t = nc.dram_tensor("ones2_t", [128, 2], F32R, kind="ExternalInput")
    sel2_t = nc.dram_tensor("sel2_t", [2, 128], F32R, kind="ExternalInput")
    id_t = nc.dram_tensor("id_t", [128, 128], F32R, kind="ExternalInput")
    y_out = nc.dram_tensor("y_out", [NQ, C], F32, kind="ExternalOutput")
    out_r = y_out.rearrange("(qt p) c -> p qt c", p=128)

    with tile.TileContext(nc) as tc:
        with (
            tc.tile_pool(name="wsb", bufs=1) as wsb,
            tc.tile_pool(name="res", bufs=1) as res,
            tc.tile_pool(name="xs", bufs=2) as xs,
            tc.tile_pool(name="work", bufs=2) as work,
            tc.tile_pool(name="pts", bufs=4) as pts,
            tc.tile_pool(name="small", bufs=2) as small,
        ):
            import contextlib

            def loop_ctx():
                if reps > 1:
                    return tc.For_i(0, reps, 1)
                return contextlib.nullcontext()

            with loop_ctx():
                # ---- weights: x streams on the sync queue; everything else
                # rides the (otherwise idle) Pool DGE queue in parallel ----
                wa_sb = wsb.tile([128, 8, 2 * R], BF16, tag="wa")
                nc.sync.dma_start(wa_sb[:], wa_t[:])
                m_sb = wsb.tile([128, 8, 128], F32R, tag="m")
                nc.gpsimd.dma_start(m_sb[:], m_t[:])
                wv_sb = wsb.tile([128, H * D], F32R, tag="wv")
                nc.gpsimd.dma_start(wv_sb[:], wv_t[:])
                ones2 = wsb.tile([128, 2], F32R, tag="ones2")
                nc.gpsimd.dma_start(ones2[:], ones2_t[:])
                sel2 = wsb.tile([2, 128], F32R, tag="sel2")
                nc.gpsimd.dma_start(sel2[:], sel2_t[:])
                wp_sb = wsb.tile([128, 8, C], BF16, tag="wp")
                nc.gpsimd.dma_start(wp_sb[:], wp_t[:])
                id_sb = wsb.tile([128, 128], F32R, tag="id")
                nc.gpsimd.dma_start(id_sb[:], id_t[:])
                xts = []
                xq = [nc.sync, nc.scalar, nc.gpsimd]
                for t8 in range(8):
                    xt = xs.tile([128, 8, 256], BF16, tag="x", bufs=4)
                    xq[t8 % 3].dma_start(xt[:],
                                         x_t[:, :, t8 * 256:(t8 + 1) * 256])
                    xts.append(xt)

                # ---- constants ----
                ones_row = wsb.tile([128, 64], F32, tag="ones_row")
                nc.vector.memset(ones_row[:], 1.0)
                eps2 = wsb.tile([2, 1], F32, tag="eps")
                nc.vector.memset(eps2[:], EPS)

                # ---- resident tensors ----
                lat_n = res.tile([128, NT], F32R, tag="lat_n")
                ybuf = res.tile([128, 8, NQ], BF16, tag="ybuf")
                osum = res.tile([128, 8, C], F32R, tag="osum")

                # ---- phase 0: fused latents + rmsnorm ----
                with tc.tile_pool(name="ps0", bufs=2, space="PSUM") as ps0:
                    lat_ps = ps0.tile([128, NT], F32, tag="lat", bufs=1)
                    for t8 in range(8):
                        sl8 = bass.ts(t8, 256)
                        for kc in range(8):
                            nc.tensor.matmul(
                                lat_ps[:, sl8],
                                wa_sb[:, kc, :],
                                xts[t8][:, kc, :],
                                start=(kc == 0), stop=(kc == 7))
                    # batch ACT ops per function so each activation table
                    # loads at most once (Square x4, Ln x4, Exp x4)
                    ssqs, lnss = [], []
                    for t4 in range(4):
                        sl = bass.ts(t4, 512)
                        sq = small.tile([128, 512], F32R, tag="sq",
                                        name=f"sq{t4}", bufs=2)
                        nc.scalar.activation(sq[:], lat_ps[:, sl], AF.Square)
                        ssq = ps0.tile([2, 512], F32, tag="aux", bufs=2)
                        nc.tensor.matmul(ssq[:], ones2[:], sq[:], start=True, stop=True)
                        ssqs.append(ssq)
                    for t4 in range(4):
                        lns = small.tile([2, 512], F32, tag="lns",
                                         name=f"lns{t4}", bufs=4)
                        nc.scalar.activation(lns[:], ssqs[t4][:], AF.Ln,
                                             bias=eps2[:], scale=1.0 / R)
                        lnss.append(lns)
                    lat_kv2 = res.tile([128, NT], F32R, tag="lat_kv2")
                    for t4 in range(4):
                        sl = bass.ts(t4, 512)
                        inv = small.tile([2, 512], F32R, tag="inv", bufs=2)
                        nc.scalar.activation(inv[:], lnss[t4][:], AF.Exp, scale=-0.5)
                        bcp0 = ps0.tile([128, 512], F32, tag="bc", bufs=2)
                        nc.tensor.matmul(bcp0[:], sel2[:], inv[:],
                                         start=True, stop=True)
                        # walrus allows at most one PSUM operand per DVE op
                        bc_sb = small.tile([128, 512], F32R, tag="bcs", bufs=2)
                        nc.vector.tensor_copy(bc_sb[:], bcp0[:])
                        nc.vector.tensor_mul(lat_n[:, sl], lat_ps[:, sl], bc_sb[:])
                        # duplicate kv-latent rows at partitions 64-127 chunk
                        # by chunk (row-group pairing for V and S_B matmuls)
                        nc.gpsimd.dma_start(lat_kv2[64:128, sl], lat_n[0:64, sl])

                # ---- phase 1: head pairs ----
                with (
                    tc.tile_pool(name="pst", bufs=2, space="PSUM") as pst,
                    tc.tile_pool(name="psy", bufs=2, space="PSUM") as psy,
                ):
                    def z_tile_and_thunks(hp):
                        """Q-side z for pair hp: two [128,512] matmuls."""
                        zpr = work.tile([128, NQ], F32R, tag="zpr",
                                        name=f"zpr{hp}")

                        def z_unit(t2):
                            # Z_pair = [M_A | M_B]^T @ L_q: one M=128 matmul
                            # produces both heads' Z (rows 0-63 A, 64-127 B)
                            sl = bass.ts(t2, 512)
                            zps = pst.tile([128, 512], F32, tag="kqv")
                            nc.tensor.matmul(zps[:],
                                             m_sb[64:128, hp, :],
                                             lat_n[64:128, sl],
                                             start=True, stop=True)
                            nc.vector.tensor_copy(zpr[:, sl], zps[:])

                        return zpr, [lambda t2=t2: z_unit(t2) for t2 in range(2)]

                    def vgroup_tile_and_thunks(g):
                        """V for head-pairs (2g, 2g+1): [128,256]-wide f32r
                        matmuls (free>=256 keeps 1 cyc/row), even/odd k-tiles
                        on opposite PE row groups."""
                        vt = work.tile([128, 16, 2, 130], F32R, tag="vt",
                                       name=f"vt{g}")
                        vt5 = vt.rearrange("p k h (s u) -> p k h s u", s=2)
                        gsl = bass.ts(g, 256)
                        thunks = []

                        def ones_unit():
                            nc.vector.tensor_copy(
                                vt5[:, :, :, :, 64:65],
                                ones_row[:, 0:1].broadcast_to([128, 16, 2, 2, 1]))

                        def v_unit(kt):
                            vps = pst.tile([128, 256], F32, tag="kqv")
                            if kt % 2 == 0:
                                nc.tensor.matmul(
                                    vps[:], lat_n[0:64, bass.ts(kt, 128)],
                                    wv_sb[0:64, gsl], start=True, stop=True)
                            else:
                                nc.tensor.matmul(
                                    vps[:], lat_kv2[64:128, bass.ts(kt, 128)],
                                    wv_sb[64:128, gsl], start=True, stop=True)
                            nc.vector.tensor_copy(
                                vt5[:, kt, :, :, 0:64],
                                vps[:].rearrange("p (h s u) -> p h s u", h=2, s=2))

                        thunks.append(ones_unit)
                        for kt in range(16):
                            thunks.append(lambda kt=kt: v_unit(kt))
                        return vt, thunks

                    def proj_units(pair, qts):
                        """Per-(qt,cc) proj for head-pair (2p, 2p+1): both
                        hps accumulate in psum (one DVE op per pair instead
                        of per hp); pair==3 sums the final four heads in psum
                        on top of an identity-preloaded partial and streams
                        the block straight from psum."""
                        units = []
                        h0, h1 = 2 * pair, 2 * pair + 1

                        def unit(qt, cc):
                            csl = bass.ts(cc, 512)
                            pj = pst.tile([128, 512], F32, tag="kqv")
                            if pair == 3:
                                # preload the pairs-0-2 partial into psum via
                                # an identity matmul, accumulate the final two
                                # head pairs on top, stream straight from psum
                                # (no DVE add in the tail)
                                nc.tensor.matmul(pj[:], id_sb[:],
                                                 osum[:, qt, csl],
                                                 start=True, stop=False)
                                nc.tensor.matmul(pj[:],
                                                 ybuf[:, h0, bass.ts(qt, 128)],
                                                 wp_sb[:, h0, csl],
                                                 start=False, stop=False)
                                nc.tensor.matmul(pj[:],
                                                 ybuf[:, h1, bass.ts(qt, 128)],
                                                 wp_sb[:, h1, csl],
                                                 start=False, stop=True)
                                ostrip = small.tile([128, 512], F32,
                                                    tag="ostrip")
                                nc.vector.tensor_copy(ostrip[:], pj[:])
                                nc.gpsimd.dma_start(out_r[:, qt, csl],
                                                    ostrip[:])
                                return
                            nc.tensor.matmul(pj[:],
                                             ybuf[:, h0, bass.ts(qt, 128)],
                                             wp_sb[:, h0, csl],
                                             start=True, stop=False)
                            nc.tensor.matmul(pj[:],
                                             ybuf[:, h1, bass.ts(qt, 128)],
                                             wp_sb[:, h1, csl],
                                             start=False, stop=True)
                            if pair == 0:
                                nc.vector.tensor_copy(osum[:, qt, csl], pj[:])
                            else:
                                nc.vector.tensor_add(osum[:, qt, csl],
                                                     osum[:, qt, csl], pj[:])

                        for qt in qts:
                            for cc in range(2):
                                units.append(lambda qt=qt, cc=cc: unit(qt, cc))
                        return units

                    filler = deque()
                    proj_ready = deque()
                    deferred = []
                    zpr0, z_th = z_tile_and_thunks(0)
                    vt_g, v_th = vgroup_tile_and_thunks(0)
                    for th in z_th:
                        th()
                    filler.extend(v_th)
                    cur_z, cur_v = zpr0, vt_g
                    for hp in range(8):
                        zpr, vt = cur_z, cur_v
                        h2 = hp % 2
                        if hp < 7:
                            cur_z, z_th = z_tile_and_thunks(hp + 1)
                            filler.extend(z_th)
                        if h2 == 1 and hp < 7:
                            cur_v, v_th = vgroup_tile_and_thunks((hp + 1) // 2)
                            filler.extend(v_th)
                        # attention: per qc a single chain; per kt one
                        # [128,1024] (A|B) psum group -> one exp; y matmuls
                        # delayed two kts (PE FIFO never blocks on exp).
                        for qc in ((1, 0) if hp == 7 else (0, 1)):
                            qsl = bass.ts(qc, 512)
                            ya = psy.tile([65, 512], F32, tag="y",
                                          name=f"ya{hp}_{qc}")
                            yb = psy.tile([65, 512], F32, tag="y",
                                          name=f"yb{hp}_{qc}")

                            def emit_y(kt, pt, ya=ya, yb=yb, h2=h2, vt=vt):
                                nc.tensor.matmul(ya[:], vt[:, kt, h2, 0:65],
                                                 pt[:, 0:512],
                                                 start=(kt == 0), stop=(kt == 15))
                                nc.tensor.matmul(yb[:], vt[:, kt, h2, 65:130],
                                                 pt[:, 512:1024],
                                                 start=(kt == 0), stop=(kt == 15))

                            def normalize(hp=hp, qsl=qsl, ya=ya, yb=yb):
                                # DVE evacuates the y psum banks (freeing
                                # them for the next chunk) and inverts the
                                # denominator rows in place; the broadcast
                                # matmuls + ybuf multiplies go to `deferred`
                                # (the next chunk's kt=4 slot) so the
                                # in-order PE queue never waits on the
                                # reciprocals
                                ysbs = []
                                for half, yp in ((0, ya), (1, yb)):
                                    ysb = small.tile([65, 512], F32R,
                                                     tag="ysb")
                                    nc.vector.tensor_copy(ysb[:], yp[:])
                                    with nc.allow_low_precision(
                                            reason="f32r softmax denominators"):
                                        nc.vector.reciprocal(ysb[64:65, :],
                                                             ysb[64:65, :])
                                    ysbs.append(ysb)

                                def norm_tail(hp=hp, qsl=qsl, ysbs=ysbs):
                                    for half, ysb in ((0, ysbs[0]),
                                                      (1, ysbs[1])):
                                        bcp = pst.tile([64, 512], F32,
                                                       tag="kqv")
                                        nc.tensor.matmul(
                                            bcp[:],
                                            ones_row[64:65, :].bitcast(F32R),
                                            ysb[64:65, :],
                                            start=True, stop=True)
                                        # half B writes partitions 64-127
                                        # directly (cross-partition-base DVE)
                                        nc.vector.tensor_mul(
                                            ybuf[64 * half:64 * half + 64,
                                                 hp, qsl],
                                            ysb[0:64, :], bcp[:])

                                deferred.append(norm_tail)

                            for kt in range(16):
                                if kt == 4 and deferred:
                                    # previous chunk's normalize tail: its
                                    # reciprocals (emitted via the pipe at
                                    # kt<=2) have finished on DVE by now, so
                                    # the broadcast matmuls won't stall the
                                    # in-order PE queue
                                    for th in deferred:
                                        th()
                                    deferred.clear()
                                if kt == 6 and proj_ready:
                                    # previous chunks' ybuf halves are
                                    # written by now; their proj work can't
                                    # stall the in-order PE queue
                                    filler.extend(proj_ready.popleft())
                                ksl = bass.ts(kt, 128)
                                st = pst.tile([128, 1024], F32, tag="st")
                                nc.tensor.matmul(st[:, 0:512],
                                                 lat_n[0:64, ksl],
                                                 zpr[0:64, qsl],
                                                 start=True, stop=True)
                                nc.tensor.matmul(st[:, 512:1024],
                                                 lat_kv2[64:128, ksl],
                                                 zpr[64:128, qsl],
                                                 start=True, stop=True)
                                pt = pts.tile([128, 1024], F32R, tag="pt")
                                nc.scalar.activation(pt[:], st[:],
                                                     AF.Exp, scale=SCALE)
                                pipe.append(
                                    lambda kt=kt, pt=pt: emit_y(kt, pt))
                                while len(pipe) > 2:
                                    pipe.popleft()()
                                if filler:
                                    filler.popleft()()
                                if hp == 0 and qc == 0 and filler:
                                    # drain the g=0 V units two per slot so
                                    # vt[kt] lands ahead of its y matmul
                                    filler.popleft()()
                            # the chunk's y tail (last 2 emit_y + normalize)
                            # drains inside the NEXT chunk's first slots, so
                            # ACT stays fed across the boundary
                            pipe.append(normalize)
                            if hp % 2 == 1:
                                proj_ready.append(
                                    proj_units(hp // 2,
                                               (0, 1, 2, 3) if qc == 0
                                               else (4, 5, 6, 7),
                                               ))
                    # ---- tail: the final chunk's normalize + proj, then two
                    # batched output DMAs on parallel queues ----
                    for th in deferred:
                        th()
                    deferred.clear()
                    while proj_ready:
                        filler.extend(proj_ready.popleft())
                    while filler:
                        filler.popleft()()
    nc.compile()
    return nc


def prep_inputs(x, w_kv_a, w_kv_b, w_q_a, w_q_b, w_proj, kv_norm_w, q_norm_w):
    """Host-side sharding/layout prep. Returns per-core input maps."""
    x = np.asarray(x, dtype=np.float32)
    w_kv_b = np.asarray(w_kv_b, dtype=np.float32) * np.asarray(kv_norm_w, np.float32)[None, :]
    w_q_b = np.asarray(w_q_b, dtype=np.float32) * np.asarray(q_norm_w, np.float32)[None, :]
    wa = np.concatenate([np.asarray(w_kv_a, np.float32),
                         np.asarray(w_q_a, np.float32)], axis=0).T   # [C, 128]
    wa_t = np.ascontiguousarray(
        wa.reshape(8, 128, 2 * R).transpose(1, 0, 2)).astype(ml_dtypes.bfloat16)
    kvb = w_kv_b.reshape(H, 2, D, R)
    wv = np.ascontiguousarray(kvb[:, 1].transpose(2, 0, 1).reshape(R, H * D))
    wv_t = np.ascontiguousarray(np.concatenate([wv, wv], axis=0))  # [128, H*D]
    # per-head folded S-matrix M_h = Wq_h^T @ Wk_h  [R, R]
    m_t = np.zeros((128, 8, 128), np.float32)
    for hp_ in range(8):
        wq_a_h = w_q_b[(2 * hp_) * D:(2 * hp_ + 1) * D, :]        # [D, R]
        wq_b_h = w_q_b[(2 * hp_ + 1) * D:(2 * hp_ + 2) * D, :]
        wk_a_h = kvb[2 * hp_, 0]                                  # [D, R]
        wk_b_h = kvb[2 * hp_ + 1, 0]
        m_t[64:128, hp_, 0:64] = wq_a_h.T @ wk_a_h
        m_t[64:128, hp_, 64:128] = wq_b_h.T @ wk_b_h
    wp = np.asarray(w_proj, np.float32).T                          # [H*D, C]
    wp_t = np.ascontiguousarray(
        wp.reshape(8, 128, C).transpose(1, 0, 2)).astype(ml_dtypes.bfloat16)

    in_maps = []
    for core in range(N_CORES):
        b, half = divmod(core, 2)
        own = x[b, half * NQ:(half + 1) * NQ]
        other = x[b, (1 - half) * NQ:(2 - half) * NQ]
        x_perm_t = np.concatenate([own, other], axis=0).T          # [C, NT]
        x3 = np.ascontiguousarray(
            x_perm_t.reshape(8, 128, NT).transpose(1, 0, 2)).astype(ml_dtypes.bfloat16)
        in_maps.append({
            "x_t": x3, "wa_t": wa_t, "m_t": m_t,
            "wv_t": wv_t, "wp_t": wp_t, "ones2_t": _ONES2,
            "sel2_t": _SEL2, "id_t": _EYE,
        })
    return in_maps


def assemble_output(results):
    out = np.empty((B, N, C), dtype=np.float32)
    for core in range(N_CORES):
        b, half = divmod(core, 2)
        out[b, half * NQ:(half + 1) * NQ] = results[core]["y_out"]
    return out


_ONES2 = np.zeros((128, 2), np.float32)
_ONES2[0:64, 0] = 1.0
_ONES2[64:128, 1] = 1.0
_SEL2 = np.zeros((2, 128), np.float32)
_SEL2[0, 0:64] = 1.0
_SEL2[1, 64:128] = 1.0
_EYE = np.eye(128, dtype=np.float32)

_NC_CACHE = {}


def kernel(**inputs) -> np.ndarray:
    if 1 not in _NC_CACHE:
        _NC_CACHE[1] = build_nc(reps=1)
    nc = _NC_CACHE[1]
    in_maps = prep_inputs(**inputs)
    res = run_bass_kernel_spmd(nc, in_maps, core_ids=list(range(N_CORES)))
    return assemble_output(res.results)



# revision 12
# speedup vs baseline: 1.3289x; 1.3289x over previous
"""Multi-Head Latent Attention Trainium2 kernel (8-core data parallel).

Sharding: pure data parallel over (batch=4) x (sequence halves=2) = 8 cores.
Each core computes the full attention output for its 1024 query tokens of one
batch, using all 2048 keys/values of that batch (KV computed redundantly per
batch pair — negligible cost). No collectives.

Per-core kernel (matmul cost = moving rows; f32r 1 cyc/row at free>=256):
  0. inputs land via few BIG DMAs (host pre-lays x/weights in partition-major
     order) — DMA issue on the sync queue costs ~2us each, so count matters.
  1. latent.T [128, 2048] = [w_kv_a; w_q_a] @ x.T; rmsnorm via ones-matmul
     sum-of-squares, inv_rms = exp(-0.5*ln(ms+eps)) on ACT (ops batched
     per-function so each activation table loads once); partition broadcast
     of inv_rms via a sel2 matmul into psum (no DMA round trip).
  2. V per GROUP of two head-pairs: [128tok, 256] f32r matmuls (1 cyc/row).
     Per head pair (A=2hp rows 0-63, B=2hp+1 rows 64-127):
     S^T = K.T_tile.T @ Q.T chunk (A/B row groups, shared 2-bank psum),
     P = exp(S*scale) one ACT op per [128,1024] tile, y_aug.T accumulated
     over k-tiles (M=65 incl. denominator row), y matmuls delayed two
     k-tiles so the PE never blocks on ACT. Softmax normalization:
     reciprocal in place on the denominator row (partition 64), broadcast
     down 64 partitions with a ones-row matmul (bases aligned), DVE multiply.
  3. proj: spread through the attention phase as filler work: per
     (head-pair, qt, cc) two psum-accumulated matmuls + one DVE accumulate
     into osum; per-(qt,cc) output DMA after the pair=3 partial. For hp=7
     the qc order is swapped so only one proj batch remains after the last
     chunk.
  ACT runs ONLY the exps (the ~290us/core softmax exp is the hard floor:
  no other engine has exp); all PSUM evacuations (z/V/ysb/ostrip) run on
  DVE, which also does reciprocals, normalize multiplies and osum
  accumulation; Pool issues weight/output DMAs.

Token order per core: [own 1024 queries, other half] so the SPMD NEFF always
reads queries at offset 0 (K/V order irrelevant to softmax).
"""
from collections import deque

import numpy as np
import ml_dtypes

import concourse.bacc as bacc
import concourse.bass as bass
import concourse.mybir as mybir
import concourse.tile as tile
from concourse.bass_utils import run_bass_kernel_spmd

F32 = mybir.dt.float32
F32R = mybir.dt.float32r
BF16 = mybir.dt.bfloat16
AF = mybir.ActivationFunctionType

B, N, C = 4, 2048, 1024
H, D, R = 16, 64, 64
NT = 2048          # kv tokens per core (full batch sequence)
NQ = 1024          # query tokens per core
EPS = 1e-6
SCALE = D ** -0.5
N_CORES = 8


def build_nc(reps: int = 1, ablate=()):
    ablate = set(ablate)
    nc = bacc.Bacc("TRN2", target_bir_lowering=False)
    # partition-major host layouts: one DMA each
    x_t = nc.dram_tensor("x_t", [128, 8, NT], BF16, kind="ExternalInput")
    wa_t = nc.dram_tensor("wa_t", [128, 8, 2 * R], BF16, kind="ExternalInput")
    m_t = nc.dram_tensor("m_t", [128, 8, 128], BF16, kind="ExternalInput")
    wv_t = nc.dram_tensor("wv_t", [128, H * D], BF16, kind="ExternalInput")
    wp_t = nc.dram_tensor("wp_t", [128, 8, C], BF16, kind="ExternalInput")
    ones2_t = nc.dram_tensor("ones2_t", [128, 2], F32R, kind="ExternalInput")
    sel2_t = nc.dram_tensor("sel2_t", [2, 128], F32R, kind="ExternalInput")
    id_t = nc.dram_tensor("id_t", [128, 128], F32R, kind="ExternalInput")
    y_out = nc.dram_tensor("y_out", [NQ, C], F32, kind="ExternalOutput")
    out_r = y_out.rearrange("(qt p) c -> p qt c", p=128)

    with tile.TileContext(nc) as tc:
        with (
            tc.tile_pool(name="wsb", bufs=1) as wsb,
            tc.tile_pool(name="res", bufs=1) as res,
            tc.tile_pool(name="xs", bufs=2) as xs,
            tc.tile_pool(name="work", bufs=2) as work,
            tc.tile_pool(name="pts", bufs=4) as pts,
            tc.tile_pool(name="small", bufs=2) as small,
        ):
            import contextlib

            def loop_ctx():
                if reps > 1:
                    return tc.For_i(0, reps, 1)
                return contextlib.nullcontext()

            with loop_ctx():
                # ---- weights: x streams on the sync queue; everything else
                # rides the (otherwise idle) Pool DGE queue in parallel ----
                wa_sb = wsb.tile([128, 8, 2 * R], BF16, tag="wa")
                nc.sync.dma_start(wa_sb[:], wa_t[:])
                m_sb = wsb.tile([128, 8, 128], BF16, tag="m")
                nc.gpsimd.dma_start(m_sb[:], m_t[:])
                wv_sb = wsb.tile([128, H * D], BF16, tag="wv")
                nc.gpsimd.dma_start(wv_sb[:], wv_t[:])
                ones2 = wsb.tile([128, 2], F32R, tag="ones2")
                nc.gpsimd.dma_start(ones2[:], ones2_t[:])
                sel2 = wsb.tile([2, 128], F32R, tag="sel2")
                nc.gpsimd.dma_start(sel2[:], sel2_t[:])
                wp_sb = wsb.tile([128, 8, C], BF16, tag="wp")
                nc.gpsimd.dma_start(wp_sb[:], wp_t[:])
                id_sb = wsb.tile([128, 128], F32R, tag="id")
                nc.gpsimd.dma_start(id_sb[:], id_t[:])
                xts = []
                xq = [nc.sync, nc.scalar, nc.gpsimd]
                for t8 in range(8):
                    xt = xs.tile([128, 8, 256], BF16, tag="x", bufs=4)
                    xq[t8 % 3].dma_start(xt[:],
                                         x_t[:, :, t8 * 256:(t8 + 1) * 256])
                    xts.append(xt)

                # ---- constants ----
                ones_row = wsb.tile([128, 64], F32, tag="ones_row")
                nc.vector.memset(ones_row[:], 1.0)
                eps2 = wsb.tile([2, 1], F32, tag="eps")
                nc.vector.memset(eps2[:], EPS)

                # ---- resident tensors ----
                lat_n = res.tile([128, NT], BF16, tag="lat_n")
                ybuf = res.tile([128, 8, NQ], BF16, tag="ybuf")
                osum = res.tile([128, 8, C], F32R, tag="osum")

                # ---- phase 0: fused latents + rmsnorm ----
                with tc.tile_pool(name="ps0", bufs=2, space="PSUM") as ps0:
                    lat_ps = ps0.tile([128, NT], F32, tag="lat", bufs=1)
                    for t8 in range(8):
                        sl8 = bass.ts(t8, 256)
                        for kc in range(8):
                            nc.tensor.matmul(
                                lat_ps[:, sl8],
                                wa_sb[:, kc, :],
                                xts[t8][:, kc, :],
                                start=(kc == 0), stop=(kc == 7))
                    # batch ACT ops per function so each activation table
                    # loads at most once (Square x4, Ln x4, Exp x4)
                    ssqs, lnss = [], []
                    for t4 in range(4):
                        sl = bass.ts(t4, 512)
                        sq = small.tile([128, 512], F32R, tag="sq",
                                        name=f"sq{t4}", bufs=2)
                        nc.scalar.activation(sq[:], lat_ps[:, sl], AF.Square)
                        ssq = ps0.tile([2, 512], F32, tag="aux", bufs=2)
                        nc.tensor.matmul(ssq[:], ones2[:], sq[:], start=True, stop=True)
                        ssqs.append(ssq)
                    for t4 in range(4):
                        lns = small.tile([2, 512], F32, tag="lns",
                                         name=f"lns{t4}", bufs=4)
                        nc.scalar.activation(lns[:], ssqs[t4][:], AF.Ln,
                                             bias=eps2[:], scale=1.0 / R)
                        lnss.append(lns)
                    lat_kv2 = res.tile([128, NT], BF16, tag="lat_kv2")
                    for t4 in range(4):
                        sl = bass.ts(t4, 512)
                        inv = small.tile([2, 512], F32R, tag="inv", bufs=2)
                        nc.scalar.activation(inv[:], lnss[t4][:], AF.Exp, scale=-0.5)
                        bcp0 = ps0.tile([128, 512], F32, tag="bc", bufs=2)
                        nc.tensor.matmul(bcp0[:], sel2[:], inv[:],
                                         start=True, stop=True)
                        # walrus allows at most one PSUM operand per DVE op
                        bc_sb = small.tile([128, 512], F32R, tag="bcs", bufs=2)
                        nc.vector.tensor_copy(bc_sb[:], bcp0[:])
                        nc.vector.tensor_mul(lat_n[:, sl], lat_ps[:, sl], bc_sb[:])
                        # duplicate kv-latent rows at partitions 64-127 chunk
                        # by chunk (row-group pairing for V and S_B matmuls)
                        nc.gpsimd.dma_start(lat_kv2[64:128, sl], lat_n[0:64, sl])

                # ---- phase 1: head pairs ----
                with (
                    tc.tile_pool(name="pst", bufs=2, space="PSUM") as pst,
                    tc.tile_pool(name="psy", bufs=2, space="PSUM") as psy,
                ):
                    def z_tile_and_thunks(hp):
                        """Q-side z for pair hp: two [128,512] matmuls."""
                        zpr = work.tile([128, NQ], BF16, tag="zpr",
                                        name=f"zpr{hp}")

                        def z_unit(t2):
                            # Z_pair = [M_A | M_B]^T @ L_q: one M=128 matmul
                            # produces both heads' Z (rows 0-63 A, 64-127 B)
                            sl = bass.ts(t2, 512)
                            zps = pst.tile([128, 512], F32, tag="kqv")
                            nc.tensor.matmul(zps[:],
                                             m_sb[64:128, hp, :],
                                             lat_n[64:128, sl],
                                             start=True, stop=True)
                            nc.vector.tensor_copy(zpr[:, sl], zps[:])

                        return zpr, [lambda t2=t2: z_unit(t2) for t2 in range(2)]

                    def vgroup_tile_and_thunks(g):
                        """V for head-pairs (2g, 2g+1): [128,256]-wide f32r
                        matmuls (free>=256 keeps 1 cyc/row), even/odd k-tiles
                        on opposite PE row groups."""
                        vt = work.tile([128, 16, 2, 130], BF16, tag="vt",
                                       name=f"vt{g}")
                        vt5 = vt.rearrange("p k h (s u) -> p k h s u", s=2)
                        gsl = bass.ts(g, 256)
                        thunks = []

                        def ones_unit():
                            nc.vector.tensor_copy(
                                vt5[:, :, :, :, 64:65],
                                ones_row[:, 0:1].broadcast_to([128, 16, 2, 2, 1]))

                        def v_unit(kt):
                            vps = pst.tile([128, 256], F32, tag="kqv")
                            if kt % 2 == 0:
                                nc.tensor.matmul(
                                    vps[:], lat_n[0:64, bass.ts(kt, 128)],
                                    wv_sb[0:64, gsl], start=True, stop=True)
                            else:
                                nc.tensor.matmul(
                                    vps[:], lat_kv2[64:128, bass.ts(kt, 128)],
                                    wv_sb[64:128, gsl], start=True, stop=True)
                            nc.vector.tensor_copy(
                                vt5[:, kt, :, :, 0:64],
                                vps[:].rearrange("p (h s u) -> p h s u", h=2, s=2))

                        thunks.append(ones_unit)
                        for kt in range(16):
                            thunks.append(lambda kt=kt: v_unit(kt))
                        return vt, thunks

                    def proj_units(pair, qts):
                        """Per-(qt,cc) proj for head-pair (2p, 2p+1): both
                        hps accumulate in psum (one DVE op per pair instead
                        of per hp); pair==3 sums the final four heads in psum
                        on top of an identity-preloaded partial and streams
                        the block straight from psum."""
                        units = []
                        h0, h1 = 2 * pair, 2 * pair + 1

                        def unit(qt, cc):
                            csl = bass.ts(cc, 512)
                            pj = pst.tile([128, 512], F32, tag="kqv")
                            if pair == 3:
                                # preload the pairs-0-2 partial into psum via
                                # an identity matmul, accumulate the final two
                                # head pairs on top, stream straight from psum
                                # (no DVE add in the tail)
                                nc.tensor.matmul(pj[:], id_sb[:],
                                                 osum[:, qt, csl],
                                                 start=True, stop=False)
                                nc.tensor.matmul(pj[:],
                                                 ybuf[:, h0, bass.ts(qt, 128)],
                                                 wp_sb[:, h0, csl],
                                                 start=False, stop=False)
                                nc.tensor.matmul(pj[:],
                                                 ybuf[:, h1, bass.ts(qt, 128)],
                                                 wp_sb[:, h1, csl],
                                                 start=False, stop=True)
                                ostrip = small.tile([128, 512], F32,
                                                    tag="ostrip")
                                nc.vector.tensor_copy(ostrip[:], pj[:])
                                nc.gpsimd.dma_start(out_r[:, qt, csl],
                                                    ostrip[:])
                                return
                            nc.tensor.matmul(pj[:],
                                             ybuf[:, h0, bass.ts(qt, 128)],
                                             wp_sb[:, h0, csl],
                                             start=True, stop=False)
                            nc.tensor.matmul(pj[:],
                                             ybuf[:, h1, bass.ts(qt, 128)],
                                             wp_sb[:, h1, csl],
                                             start=False, stop=True)
                            if pair == 0:
                                nc.vector.tensor_copy(osum[:, qt, csl], pj[:])
                            else:
                                nc.vector.tensor_add(osum[:, qt, csl],
                                                     osum[:, qt, csl], pj[:])

                        for qt in qts:
                            for cc in range(2):
                                units.append(lambda qt=qt, cc=cc: unit(qt, cc))
                        return units

                    filler = deque()
                    proj_ready = deque()
                    deferred = []
                    # y matmuls + normalize, delayed 2 slots; carried ACROSS
                    # chunks so ACT never bubbles at chunk boundaries (the
                    # old chunk's y tail runs inside the next chunk's first
                    # slots)
                    pipe = deque()
                    zpr0, z_th = z_tile_and_thunks(0)
                    vt_g, v_th = vgroup_tile_and_thunks(0)
                    for th in z_th:
                        th()
                    filler.extend(v_th)
                    cur_z, cur_v = zpr0, vt_g
                    for hp in range(8):
                        zpr, vt = cur_z, cur_v
                        h2 = hp % 2
                        if hp < 7:
                            cur_z, z_th = z_tile_and_thunks(hp + 1)
                            filler.extend(z_th)
                        if h2 == 1 and hp < 7:
                            cur_v, v_th = vgroup_tile_and_thunks((hp + 1) // 2)
                            filler.extend(v_th)
                        # attention: per qc a single chain; per kt one
                        # [128,1024] (A|B) psum group -> one exp; y matmuls
                        # delayed two kts (PE FIFO never blocks on exp).
                        for qc in ((1, 0) if hp == 7 else (0, 1)):
                            qsl = bass.ts(qc, 512)
                            ya = psy.tile([65, 512], F32, tag="y",
                                          name=f"ya{hp}_{qc}")
                            yb = psy.tile([65, 512], F32, tag="y",
                                          name=f"yb{hp}_{qc}")

                            def emit_y(kt, pt, ya=ya, yb=yb, h2=h2, vt=vt):
                                nc.tensor.matmul(ya[:], vt[:, kt, h2, 0:65],
                                                 pt[:, 0:512],
                                                 start=(kt == 0), stop=(kt == 15))
                                nc.tensor.matmul(yb[:], vt[:, kt, h2, 65:130],
                                                 pt[:, 512:1024],
                                                 start=(kt == 0), stop=(kt == 15))

                            def normalize(hp=hp, qsl=qsl, ya=ya, yb=yb):
                                # DVE evacuates the y psum banks (freeing
                                # them for the next chunk) and inverts the
                                # denominator rows in place; the broadcast
                                # matmuls + ybuf multiplies go to `deferred`
                                # (the next chunk's kt=4 slot) so the
                                # in-order PE queue never waits on the
                                # reciprocals
                                ysbs = []
                                for half, yp in ((0, ya), (1, yb)):
                                    ysb = small.tile([65, 512], F32R,
                                                     tag="ysb")
                                    nc.vector.tensor_copy(ysb[:], yp[:])
                                    with nc.allow_low_precision(
                                            reason="f32r softmax denominators"):
                                        nc.vector.reciprocal(ysb[64:65, :],
                                                             ysb[64:65, :])
                                    ysbs.append(ysb)

                                def norm_tail(hp=hp, qsl=qsl, ysbs=ysbs):
                                    for half, ysb in ((0, ysbs[0]),
                                                      (1, ysbs[1])):
                                        bcp = pst.tile([64, 512], F32,
                                                       tag="kqv")
                                        nc.tensor.matmul(
                                            bcp[:],
                                            ones_row[64:65, :].bitcast(F32R),
                                            ysb[64:65, :],
                                            start=True, stop=True)
                                        # half B writes partitions 64-127
                                        # directly (cross-partition-base DVE)
                                        nc.vector.tensor_mul(
                                            ybuf[64 * half:64 * half + 64,
                                                 hp, qsl],
                                            ysb[0:64, :], bcp[:])

                                deferred.append(norm_tail)

                            for kt in range(16):
                                if kt == 4 and deferred:
                                    # previous chunk's normalize tail: its
                                    # reciprocals (emitted via the pipe at
                                    # kt<=2) have finished on DVE by now, so
                                    # the broadcast matmuls won't stall the
                                    # in-order PE queue
                                    for th in deferred:
                                        th()
                                    deferred.clear()
                                if kt == 6 and proj_ready:
                                    # previous chunks' ybuf halves are
                                    # written by now; their proj work can't
                                    # stall the in-order PE queue
                                    filler.extend(proj_ready.popleft())
                                ksl = bass.ts(kt, 128)
                                st = pst.tile([128, 1024], F32, tag="st")
                                nc.tensor.matmul(st[:, 0:512],
                                                 lat_n[0:64, ksl],
                                                 zpr[0:64, qsl],
                                                 start=True, stop=True)
                                nc.tensor.matmul(st[:, 512:1024],
                                                 lat_kv2[64:128, ksl],
                                                 zpr[64:128, qsl],
                                                 start=True, stop=True)
                                pt = pts.tile([128, 1024], BF16, tag="pt")
                                nc.scalar.activation(pt[:], st[:],
                                                     AF.Exp, scale=SCALE)
                                pipe.append(
                                    lambda kt=kt, pt=pt, e=emit_y: e(kt, pt))
                                while len(pipe) > 2:
                                    pipe.popleft()()
                                if filler:
                                    filler.popleft()()
                                if hp == 0 and qc == 0 and filler:
                                    # drain the g=0 V units two per slot so
                                    # vt[kt] lands ahead of its y matmul
                                    filler.popleft()()
                            # the chunk's y tail (last 2 emit_y + normalize)
                            # drains inside the NEXT chunk's first slots, so
                            # ACT stays fed across the boundary
                            pipe.append(normalize)
                            if hp % 2 == 1:
                                proj_ready.append(
                                    proj_units(hp // 2,
                                               (0, 1, 2, 3) if qc == 0
                                               else (4, 5, 6, 7),
                                               ))
                    # ---- tail: drain the y pipe (last 2 emit_y + final
                    # normalize), then the final norm tails + proj ----
                    while pipe:
                        pipe.popleft()()
                    for th in deferred:
                        th()
                    deferred.clear()
                    while proj_ready:
                        filler.extend(proj_ready.popleft())
                    while filler:
                        filler.popleft()()
    nc.compile()
    return nc


def prep_inputs(x, w_kv_a, w_kv_b, w_q_a, w_q_b, w_proj, kv_norm_w, q_norm_w):
    """Host-side sharding/layout prep. Returns per-core input maps."""
    x = np.asarray(x, dtype=np.float32)
    w_kv_b = np.asarray(w_kv_b, dtype=np.float32) * np.asarray(kv_norm_w, np.float32)[None, :]
    w_q_b = np.asarray(w_q_b, dtype=np.float32) * np.asarray(q_norm_w, np.float32)[None, :]
    wa = np.concatenate([np.asarray(w_kv_a, np.float32),
                         np.asarray(w_q_a, np.float32)], axis=0).T   # [C, 128]
    wa_t = np.ascontiguousarray(
        wa.reshape(8, 128, 2 * R).transpose(1, 0, 2)).astype(ml_dtypes.bfloat16)
    kvb = w_kv_b.reshape(H, 2, D, R)
    wv = np.ascontiguousarray(kvb[:, 1].transpose(2, 0, 1).reshape(R, H * D))
    wv_t = np.ascontiguousarray(np.concatenate([wv, wv], axis=0)).astype(ml_dtypes.bfloat16)  # [128, H*D]
    # per-head folded S-matrix M_h = Wq_h^T @ Wk_h  [R, R]
    m_t = np.zeros((128, 8, 128), np.float32)  # cast to bf16 below
    for hp_ in range(8):
        wq_a_h = w_q_b[(2 * hp_) * D:(2 * hp_ + 1) * D, :]        # [D, R]
        wq_b_h = w_q_b[(2 * hp_ + 1) * D:(2 * hp_ + 2) * D, :]
        wk_a_h = kvb[2 * hp_, 0]                                  # [D, R]
        wk_b_h = kvb[2 * hp_ + 1, 0]
        m_t[64:128, hp_, 0:64] = wq_a_h.T @ wk_a_h
        m_t[64:128, hp_, 64:128] = wq_b_h.T @ wk_b_h
    m_t = m_t.astype(ml_dtypes.bfloat16)
    wp = np.asarray(w_proj, np.float32).T                          # [H*D, C]
    wp_t = np.ascontiguousarray(
        wp.reshape(8, 128, C).transpose(1, 0, 2)).astype(ml_dtypes.bfloat16)

    in_maps = []
    for core in range(N_CORES):
        b, half = divmod(core, 2)
        own = x[b, half * NQ:(half + 1) * NQ]
        other = x[b, (1 - half) * NQ:(2 - half) * NQ]
        x_perm_t = np.concatenate([own, other], axis=0).T          # [C, NT]
        x3 = np.ascontiguousarray(
            x_perm_t.reshape(8, 128, NT).transpose(1, 0, 2)).astype(ml_dtypes.bfloat16)
        in_maps.append({
            "x_t": x3, "wa_t": wa_t, "m_t": m_t,
            "wv_t": wv_t, "wp_t": wp_t, "ones2_t": _ONES2,
            "sel2_t": _SEL2, "id_t": _EYE,
        })
    return in_maps


def assemble_output(results):
    out = np.empty((B, N, C), dtype=np.float32)
    for core in range(N_CORES):
        b, half = divmod(core, 2)
        out[b, half * NQ:(half + 1) * NQ] = results[core]["y_out"]
    return out


_ONES2 = np.zeros((128, 2), np.float32)
_ONES2[0:64, 0] = 1.0
_ONES2[64:128, 1] = 1.0
_SEL2 = np.zeros((2, 128), np.float32)
_SEL2[0, 0:64] = 1.0
_SEL2[1, 64:128] = 1.0
_EYE = np.eye(128, dtype=np.float32)

_NC_CACHE = {}


def kernel(**inputs) -> np.ndarray:
    if 1 not in _NC_CACHE:
        _NC_CACHE[1] = build_nc(reps=1)
    nc = _NC_CACHE[1]
    in_maps = prep_inputs(**inputs)
    res = run_bass_kernel_spmd(nc, in_maps, core_ids=list(range(N_CORES)))
    return assemble_output(res.results)

